# revision 22
# baseline (speedup 1.0000x reference)
"""Trainium2 Bass kernel for the dense_cnn problem (1x1 conv + BN + SiLU ->
attention-weighted dynamic 3x3 conv with instance norm), data-parallel over
batch across 8 NeuronCores.

Self-contained: hardcodes all shapes. kernel(**inputs) takes full inputs and
returns the full output.

v3 design (per core, 2 samples, single-pass critical-path minimized):
  The For_i timing loop has an all-engine barrier at its back edge, so each
  iteration is a full drain; the schedule minimizes one pass end-to-end.
  Per-sample pipeline: ph1(A) -> attn(A) -> ph3(A) -> ph5(A) with ALL of
  sample B's pre-work (conv1, attention, weight aggregation) emitted as
  fillers at ph5(A)'s PSUM-group boundaries so it hides under ph5(A)'s
  ~96us of PE time; ph5(B) then starts immediately.

  ph1: conv1 matmuls -> ACT Silu written DIRECTLY into the mu-padded ypad
       interior (no staging copy); DVE bn_stats on the strided interior view
       (per-row stats, one bn_aggr over 80 row-groups).
  Instance norm folded into the dynamic conv (weights scaled by rstd on the
       PSUM->SBUF copy, border ring holds mu, bias correction via tap-sum).
  ph5: 18 accumulating matmuls per flat PSUM tile; drains split between ACT
       (Identity w/ per-partition scale+bias) and DVE (tensor_scalar
       mult+add) so PSUM slots free fast enough for zero PE stalls with 5
       rotating bank-aligned slots; per-group DMA out.
  ACT tables: Sqrt and Exp preloaded via zero-dep dummy ops placed right
       after the silu batches so table loads overlap bn_stats/aggr latency.
  PSUM budget (8 banks): ph5 cps 5 x [128,512]f32; ph3/attn/ph1A shared
       pool 2 x [128,2,256]f32; ph1B 1 x [128,512]f32.
"""

import os

import numpy as np

os.environ.setdefault("BASS_NEVER_TRACE", "1")  # no NTFF hook in this container

EPS = 1e-5
B, C1, C2, K, H, W = 16, 128, 256, 4, 80, 80
NCORES = 8
BL = B // NCORES          # samples per core
RT = 6                    # rows per spatial tile (480 cols <= 512 PSUM bank)
ROWS = [(r, min(RT, H - r)) for r in range(0, H, RT)]   # 13x6 + 1x2
NTILES = len(ROWS)
HP, WP = H + 2, W + 2     # padded 82x82
SGS = [(0, 4), (4, 4), (8, 4), (12, 2)]  # psum accumulation groups over tiles
LDW_OPT = os.environ.get("LDW_OPT", "0") == "1"

_CACHE = {}


def _patch_ldw_opt():
    """Flip walrus's --enable-ldw-opt (redundant load-weight elimination) on
    for our NEFF compiles."""
    import concourse.bass_utils as bu

    if getattr(bu, "_ldw_opt_patched", False):
        return
    orig = bu.run_command

    def patched(argv, **kw):
        if isinstance(argv, list):
            argv = ["--enable-ldw-opt=true" if a == "--enable-ldw-opt=false" else a
                    for a in argv]
        return orig(argv, **kw)

    bu.run_command = patched
    bu._ldw_opt_patched = True


def _build_nc(reps=1, loop_n=None):
    from contextlib import ExitStack

    import concourse.bass as bass
    import concourse.mybir as mybir
    import concourse.tile as tile
    from concourse import bacc

    dt = mybir.dt
    AF = mybir.ActivationFunctionType
    OP = mybir.AluOpType
    f32 = dt.float32
    bf16 = dt.bfloat16

    if LDW_OPT:
        _patch_ldw_opt()

    nc = bacc.Bacc("TRN2", target_bir_lowering=False, debug=False)

    def inp(name, shape, dtype=f32):
        return nc.dram_tensor(name, shape, dtype, kind="ExternalInput").ap()

    x_d = inp("x_sh", [BL, C1, H, W], bf16)
    c1wT_d = inp("conv1wT", [C1, C2], bf16)
    # packed consts, one DMA: cols 0:8 vec8 (bns,bnb,s1,b1 pairs), 8:306 attn
    # consts (fc2br, inv32, dybr, mask32, fc1wT), 306:435 fc2wTr+fc1b (rows 0:4)
    consts_d = inp("consts2", [128, 435])
    # [(k,i_sub), j, tap(9 + tapsum), o]
    dyw_d = inp("dywsb", [128, 8, 10, C2], bf16)
    out_d = nc.dram_tensor("out_sh", [BL, C2, H, W], bf16, kind="ExternalOutput").ap()

    with tile.TileContext(nc) as tc, ExitStack() as ctx:
        pool = lambda name, bufs, **kw: ctx.enter_context(
            tc.tile_pool(name=name, bufs=bufs, **kw)
        )
        const_p = pool("consts", 1)
        ypad_p = pool("ypad", 1)
        aggw_p = pool("aggw", 1)
        stats_p = pool("stats", 1)
        vec_p = pool("vec", 2)
        pers_p = pool("pers", 1)   # per-sample tensors alive until ph5(b)
        bd_p = pool("bd", 1)
        xin_p = pool("xin", 1)
        stage_p = pool("stage", 4)
        ostage_p = pool("ostage", 2)
        # PSUM: 6 + 2 = 8 banks. All of ph1/attn/ph3 for BOTH samples share
        # the 2-slot aps pool (their emission is strictly sequential), so ph5
        # gets 6 rotating slots — drain deadlines stretch to ~2 group-times.
        cps_p = pool("cps", 6, space="PSUM")
        aps_p = pool("aps", 2, space="PSUM")

        # --- const loads; emission order == SP DMA issue order ---
        c1wT = const_p.tile([C1, C2], bf16, tag="c1wT", name="c1wT")
        nc.sync.dma_start(c1wT[:], c1wT_d)
        consts2 = const_p.tile([128, 435], f32, tag="consts2", name="consts2")
        nc.sync.dma_start(consts2[:], consts_d)
        dyw = const_p.tile([128, 8, 10, C2], bf16, tag="dyw", name="dyw")
        for h in range(2):
            nc.sync.dma_start(dyw[:, 4 * h : 4 * h + 4, :, :],
                              dyw_d[:, 4 * h : 4 * h + 4, :, :])

        bns = consts2[:, 0:2]
        bnb = consts2[:, 2:4]
        s1c = consts2[:, 4:6]
        b1c = consts2[:, 6:8]
        fc2br = consts2[:, 8:9]
        inv32 = consts2[:, 9:10]
        dybr = consts2[:, 10 : 10 + C2]
        mask32 = consts2[:, 266:298]
        fc1wT = consts2[:, 298:306]
        fc2wTr = consts2[0:K, 306:434]
        fc1b = consts2[0:K, 434:435]

        ypads = []
        for nm in ("ypadA", "ypadB"):
            yp = ypad_p.tile([128, 2, HP, WP], bf16, tag=nm, name=nm)
            ypads.append(yp)
        epsc = const_p.tile([128, 1], f32, tag="epsc",
                            name="epsc_ldw" if LDW_OPT else "epsc")
        nc.vector.memset(epsc[:], EPS)
        onesT = const_p.tile([128, 128], f32, tag="onesT", name="onesT")
        nc.vector.memset(onesT[:], 1.0)
        ones1 = onesT[0:1, :]
        ones82 = onesT
        dumt = const_p.tile([128, 1], f32, tag="dumt", name="dumt")

        def dummy_act(func):
            """Zero-dependency ACT op to trigger a table load early (overlaps
            the load with other engines' latency instead of stalling a
            consumer)."""
            nc.scalar.activation(dumt[0:1, 0:1], epsc[0:1, 0:1], func)

        def xin_dma(b, xt):
            for h in range(4):
                nc.gpsimd.dma_start(xt[:, 20 * h : 20 * h + 20, :],
                                    x_d[b, :, 20 * h : 20 * h + 20, :])

        def conv_tiles(b, xt, stats, mv, tiles, aggr_after=False):
            """conv1 matmul -> silu into a contiguous stage tile (bn_stats
            needs a flat view; HW emits exactly one 6-stat group per op) ->
            gpsimd copy into the ypad interior."""
            yp = ypads[b]
            for c, t in tiles:
                r0, rn = ROWS[t]
                ps = aps_p.tile([128, 2, 256], f32, tag="aps", name="cps1")
                psf = ps.rearrange("p a b -> p (a b)")
                nc.tensor.matmul(
                    psf[:, : rn * W], c1wT[:, 128 * c : 128 * (c + 1)],
                    xt[:, r0 : r0 + rn, :], start=True, stop=True,
                )
                st = stage_p.tile([128, RT, W], bf16, tag="stage", name="stage")
                nc.scalar.activation(
                    st[:, :rn, :], psf[:, : rn * W],
                    AF.Silu, bias=bnb[:, c : c + 1], scale=bns[:, c : c + 1],
                )
                nc.vector.bn_stats(
                    stats[c][:, t, :],
                    st[:, :rn, :].rearrange("p a b -> p (a b)"),
                )
                nc.gpsimd.tensor_copy(
                    yp[:, c, r0 + 1 : r0 + rn + 1, 1 : W + 1], st[:, :rn, :]
                )
            if aggr_after:
                for c in range(2):
                    nc.vector.bn_aggr(mv[:, c, :], stats[c][:])

        def borders(b, mv):
            """Fill ypad border ring with mu_i so padding is zero in
            normalized space."""
            yp = ypads[b]
            for c in range(2):
                mu = mv[:, c, 0:1]
                nc.vector.tensor_scalar_mul(yp[:, c, 0, :], ones82[:, 0:WP], mu)
                nc.vector.tensor_scalar_mul(yp[:, c, HP - 1, :], ones82[:, 0:WP], mu)
                nc.vector.tensor_scalar_mul(yp[:, c, 1 : H + 1, 0], ones82[:, 0:H], mu)
                nc.vector.tensor_scalar_mul(yp[:, c, 1 : H + 1, WP - 1], ones82[:, 0:H], mu)

        def attn_fc1(b, mv, pers):
            """fc1 matmul + relu. Exp table must already be loaded/loading."""
            aT = aps_p.tile([128, 2, 256], f32, tag="aps", name="a_ps")
            a_ps = aT[0:K, 0, 0:1]
            for c in range(2):
                nc.tensor.matmul(
                    a_ps, fc1wT[:, K * c : K * (c + 1)], mv[:, c, 0:1],
                    start=(c == 0), stop=(c == 1),
                )
            a_sb = vec_p.tile([128, 1], f32, tag="a_sb", name="a_sb")
            nc.scalar.activation(a_sb[0:K, 0:1], a_ps, AF.Relu, bias=fc1b[:])
            pers["a_sb"] = a_sb

        def attn_fc2(b, pers):
            zT = aps_p.tile([128, 2, 256], f32, tag="aps", name="z_ps")
            z_ps = zT[:, 0, 0:1]
            nc.tensor.matmul(z_ps, fc2wTr[:], pers["a_sb"][0:K, 0:1],
                             start=True, stop=True)
            e_bc = pers["e_bc"]
            nc.scalar.activation(e_bc[:], z_ps, AF.Exp, bias=fc2br[:])
            nc.vector.tensor_scalar_mul(pers["bd"][:], mask32[:], e_bc[:, 0:1])

        def attn_S(b, pers):
            sT = aps_p.tile([128, 2, 256], f32, tag="aps", name="S_ps")
            S_ps = sT[0:1, 0, 0:1]
            nc.tensor.matmul(S_ps, inv32[:, 0:1], pers["e_bc"][:, 0:1],
                             start=True, stop=True)
            rS = vec_p.tile([128, 1], f32, tag="rS", name="rS")
            nc.vector.reciprocal(rS[0:1, 0:1], S_ps)
            pers["rS"] = rS

        def attn_rb(b, pers):
            rT = aps_p.tile([128, 2, 256], f32, tag="aps", name="rb_ps")
            rb_ps = rT[:, 0, 0:1]
            nc.tensor.matmul(rb_ps, ones1[:], pers["rS"][0:1, 0:1],
                             start=True, stop=True)
            s1S = pers["s1S"]
            for oc in range(2):
                nc.vector.tensor_scalar_mul(s1S[:, oc : oc + 1], rb_ps,
                                            s1c[:, oc : oc + 1])

        def attn_aggb(b, pers):
            gT = aps_p.tile([128, 2, 256], f32, tag="aps", name="aggb_ps")
            aggb_ps = gT[:, 0:2, 0:1]
            for oc in range(2):
                nc.tensor.matmul(
                    aggb_ps[:, oc, :], dybr[:, 128 * oc : 128 * (oc + 1)],
                    pers["e_bc"][:, 0:1], start=True, stop=True,
                )
            aggb_sb = vec_p.tile([128, 2], f32, tag="aggb_sb", name="aggb_sb")
            nc.vector.tensor_copy(aggb_sb[:], aggb_ps[:, :, 0])
            pers["aggb_sb"] = aggb_sb
            dummy_act(AF.Sqrt)

        def attn_rstd(b, mv, pers):
            """Sqrt table must be loaded/loading (dummy in attn_aggb)."""
            std = vec_p.tile([128, 2], f32, tag="std", name="std")
            nc.scalar.activation(std[:], mv[:, :, 1], AF.Sqrt, bias=epsc[:])
            rstd = pers["rstd"]
            nc.vector.reciprocal(rstd[:], std[:])
            nc.vector.tensor_copy(pers["mean_bf"][:], mv[:, :, 0])

        def attention(b, mv, pers):
            attn_fc1(b, mv, pers)
            attn_fc2(b, pers)
            attn_S(b, pers)
            attn_rb(b, pers)
            attn_aggb(b, pers)
            attn_rstd(b, mv, pers)

        def ph3_pair(b, pers, c, t0):
            """One tap-pair of aggW = blockdiag(e) @ dyw, scaled by rstd on
            the PSUM->SBUF copy."""
            aggW, bd, rstd = pers["aggW"], pers["bd"], pers["rstd"]
            agps = aps_p.tile([128, 2, C2], f32, tag="aps", name="agps")
            for jj in range(4):
                nc.tensor.matmul(
                    agps[32 * jj : 32 * jj + 32, :, :],
                    bd[:],
                    dyw[:, 4 * c + jj, t0 : t0 + 2, :],
                    start=True, stop=True,
                    tile_position=(0, 32 * jj),
                )
            nc.vector.tensor_scalar_mul(
                aggW[:, c, t0 : t0 + 2, :], agps[:],
                rstd[:, c : c + 1],
            )

        def ph3(b, pers, cs=(0, 1)):
            for c in cs:
                for t0 in range(0, 10, 2):
                    ph3_pair(b, pers, c, t0)

        def ph3_bias(b, pers):
            """B_o = sum_i mu_i * (rstd-scaled tap-sum), then
            cbias = (aggb - B) * s1S + b1."""
            aggW, mean_bf = pers["aggW"], pers["mean_bf"]
            bT = aps_p.tile([128, 2, 256], f32, tag="aps", name="B_ps")
            B_ps = bT[:, 0:2, 0:1]
            for oc in range(2):
                for ci in range(2):
                    nc.tensor.matmul(
                        B_ps[:, oc, :],
                        aggW[:, ci, 9, 128 * oc : 128 * (oc + 1)],
                        mean_bf[:, ci : ci + 1],
                        start=(ci == 0), stop=(ci == 1),
                    )
            B_sb = vec_p.tile([128, 2], f32, tag="B_sb", name="B_sb")
            nc.vector.tensor_copy(B_sb[:], B_ps[:, :, 0])
            t4 = vec_p.tile([128, 2], f32, tag="t4", name="t4")
            nc.vector.tensor_sub(t4[:], pers["aggb_sb"][:], B_sb[:])
            cbias = pers["cbias"]
            for oc in range(2):
                nc.vector.scalar_tensor_tensor(
                    cbias[:, oc : oc + 1], t4[:, oc : oc + 1],
                    pers["s1S"][:, oc : oc + 1], b1c[:, oc : oc + 1],
                    op0=OP.mult, op1=OP.add,
                )

        def ph5(b, pers, fill_map=None):
            """dynamic 3x3 conv; drains split ACT/DVE; per-group DMA out.
            fill_map maps a global weight-pass index (0..143: 18 passes per
            group, 8 groups oc-major) to closures emitted after that pass —
            used to spread the other sample's pre-work through this phase
            with enough dependency slack that the PE never stalls."""
            aggW, s1S, cbias = pers["aggW"], pers["s1S"], pers["cbias"]
            fill_map = fill_map or {}
            pi = 0
            for oc in range(2):
                stage = ostage_p.tile([128, H, W], bf16, tag="ostage", name="ostage")
                for g0, gn in SGS:
                    pss = [cps_p.tile([128, 512], f32, tag="cps", name="cps")
                           for _ in range(gn)]
                    first, last = (0, 0, 0), (1, 2, 2)
                    for c in range(2):
                        for ty in range(3):
                            for tx in range(3):
                                lhsT = aggW[:, c, 3 * ty + tx,
                                            128 * oc : 128 * (oc + 1)]
                                st = (c, ty, tx) == first
                                sp = (c, ty, tx) == last
                                for ti in range(gn):
                                    r0, rn = ROWS[g0 + ti]
                                    rhs = ypads[b][:, c, r0 + ty : r0 + ty + rn,
                                                   tx : tx + W]
                                    nc.tensor.matmul(
                                        pss[ti][:, : rn * W], lhsT, rhs,
                                        start=st, stop=sp,
                                    )
                                for f in fill_map.get(pi, ()):
                                    f()
                                pi += 1
                    for ti in range(gn):
                        r0, rn = ROWS[g0 + ti]
                        if ti % 2 == 0:
                            nc.scalar.activation(
                                stage[:, r0 : r0 + rn, :], pss[ti][:, : rn * W],
                                AF.Identity,
                                bias=cbias[:, oc : oc + 1],
                                scale=s1S[:, oc : oc + 1],
                            )
                        else:
                            nc.vector.tensor_scalar(
                                stage[:, r0 : r0 + rn, :], pss[ti][:, : rn * W],
                                s1S[:, oc : oc + 1], cbias[:, oc : oc + 1],
                                op0=OP.mult, op1=OP.add,
                            )
                    gr0 = ROWS[g0][0]
                    grn = sum(ROWS[g0 + i][1] for i in range(gn))
                    nc.gpsimd.dma_start(
                        out_d[b, 128 * oc : 128 * (oc + 1), gr0 : gr0 + grn, :],
                        stage[:, gr0 : gr0 + grn, :],
                    )

        def make_pers(b):
            sfx = "AB"[b]
            pers = {
                "aggW": aggw_p.tile([128, 2, 10, C2], bf16, tag=f"aggW{sfx}",
                                    name=f"aggW{sfx}"),
                "rstd": pers_p.tile([128, 2], f32, tag=f"rstd{sfx}", name=f"rstd{sfx}"),
                "mean_bf": pers_p.tile([128, 2], bf16, tag=f"meanbf{sfx}",
                                       name=f"meanbf{sfx}"),
                "e_bc": pers_p.tile([128, 1], f32, tag=f"ebc{sfx}", name=f"ebc{sfx}"),
                "bd": bd_p.tile([128, 32], bf16, tag=f"bd{sfx}", name=f"bd{sfx}"),
                "s1S": pers_p.tile([128, 2], f32, tag=f"s1S{sfx}", name=f"s1S{sfx}"),
                "cbias": pers_p.tile([128, 2], f32, tag=f"cbias{sfx}",
                                     name=f"cbias{sfx}"),
            }
            return pers

        # --- schedule: one barrier-bounded pass over 2 samples ---
        def body(prefetch=False, xts=None):
            for _rep in range(reps):
                statsA = [stats_p.tile([128, NTILES, 6], f32, tag=f"statsA{c}",
                                       name=f"statsA{c}") for c in range(2)]
                statsB = [stats_p.tile([128, NTILES, 6], f32, tag=f"statsB{c}",
                                       name=f"statsB{c}") for c in range(2)]
                mvA = vec_p.tile([128, 2, 2], f32, tag="mvA", name="mvA")
                mvB = vec_p.tile([128, 2, 2], f32, tag="mvB", name="mvB")
                persA = make_pers(0)
                persB = make_pers(1)
                if xts is None:
                    xtA = xin_p.tile([C1, H, W], bf16, tag="xtA", name="xtA")
                    xtB = xin_p.tile([C1, H, W], bf16, tag="xtB", name="xtB")
                else:
                    xtA, xtB = xts
                if not prefetch:
                    xin_dma(0, xtA)
                    xin_dma(1, xtB)

                # ph1(A): full conv1+silu+stats, then aggr
                tilesA = [(c, t) for c in range(2) for t in range(NTILES)]
                conv_tiles(0, xtA, statsA, mvA, tilesA, aggr_after=True)
                dummy_act(AF.Exp)    # load exp table during stats/aggr window
                attention(0, mvA, persA)
                borders(0, mvA)
                ph3(0, persA)
                ph3_bias(0, persA)
                dummy_act(AF.Silu)   # load silu table back for sample B

                # sample B pre-work spread through ph5(A) at weight-pass
                # granularity (one pass ~= 0.8us of PE work); each closure's
                # cross-engine deps resolve >=2 passes earlier, so the PE
                # stream never blocks on ACT/DVE.
                tB = [(c, t) for c in range(2) for t in range(NTILES)]
                fill = {}

                def at(p, f):
                    fill.setdefault(p, []).append(f)

                def fB(i):
                    def f():
                        conv_tiles(1, xtB, statsB, mvB, [tB[i]],
                                   aggr_after=(i == 27))
                        if i == 27:
                            dummy_act(AF.Exp)
                    return f

                # Keep closures out of each group's last 3 passes: a closure's
                # ACT/DVE ops emitted at a group tail land in the engine
                # streams ahead of that group's PSUM drains and delay them,
                # stalling the next group's slot reuse.
                slots = [p for p in range(144) if p % 18 < 15 and p % 2 == 1]
                for i in range(28):
                    at(slots[i], fB(i))                      # within oc0 groups
                at(slots[29], lambda: attn_fc1(1, mvB, persB))
                at(slots[31], lambda: attn_fc2(1, persB))
                at(slots[33], lambda: attn_S(1, persB))
                at(slots[35], lambda: attn_rb(1, persB))
                at(slots[37], lambda: attn_aggb(1, persB))
                at(slots[39], lambda: (attn_rstd(1, mvB, persB), borders(1, mvB)))
                for i in range(5):
                    at(slots[41 + i],
                       (lambda t0: lambda: ph3_pair(1, persB, 0, t0))(2 * i))
                for i in range(5):
                    at(slots[46 + i],
                       (lambda t0: lambda: ph3_pair(1, persB, 1, t0))(2 * i))

                def f_bias():
                    ph3_bias(1, persB)
                    dummy_act(AF.Silu)

                at(slots[52], f_bias)

                ph5(0, persA, fill_map=fill)
                ph5(1, persB)
                if prefetch:
                    # next iteration's inputs land during ph5; the input data
                    # is identical every iteration, so this is exact
                    xin_dma(0, xtA)
                    xin_dma(1, xtB)

        if loop_n is None:
            body()
        else:
            # prologue: first iteration's inputs + silu table; the body reads
            # and end-of-body-rewrites these same tiles each iteration
            pxtA = xin_p.tile([C1, H, W], bf16, tag="xtA", name="xtA")
            pxtB = xin_p.tile([C1, H, W], bf16, tag="xtB", name="xtB")
            xin_dma(0, pxtA)
            xin_dma(1, pxtB)
            dummy_act(AF.Silu)
            with tc.For_i(0, loop_n, 1):
                body(prefetch=True, xts=(pxtA, pxtB))

    if not nc.is_finalized():
        nc.finalize()
    return nc


def _host_prep(inputs):
    f = np.float32
    conv1_w = np.asarray(inputs["conv1_w"], f)
    bns = (np.asarray(inputs["bn_g"], f) / np.sqrt(np.asarray(inputs["bn_v"], f) + EPS))
    bnb = np.asarray(inputs["bn_b"], f) - np.asarray(inputs["bn_m"], f) * bns
    s1 = (np.asarray(inputs["bn1_g"], f) / np.sqrt(np.asarray(inputs["bn1_v"], f) + EPS))
    b1 = np.asarray(inputs["bn1_b"], f) - np.asarray(inputs["bn1_m"], f) * s1
    fc1_w = np.asarray(inputs["fc1_w"], f)
    fc2_w = np.asarray(inputs["fc2_w"], f)
    dy_w = np.asarray(inputs["dy_w"], f)
    dy_b = np.asarray(inputs["dy_b"], f)

    import ml_dtypes

    bf = ml_dtypes.bfloat16
    t = dy_w.transpose(0, 2, 3, 4, 1)              # [k, i, ty, tx, o]
    t = t.reshape(K, 8, 32, 3, 3, C2)              # [k, j, i_sub, ty, tx, o]
    t9 = t.transpose(0, 2, 1, 3, 4, 5).reshape(128, 8, 9, C2)
    dyw10 = np.concatenate([t9, t9.sum(axis=2, keepdims=True)], axis=2)
    dyw_sb = np.ascontiguousarray(dyw10.astype(bf))
    mask = np.zeros((128, 32), f)
    mask[np.arange(128), np.arange(128) % 32] = 1.0
    ks = np.arange(128) // 32
    vec8 = np.concatenate(
        [bns.reshape(2, 128).T, bnb.reshape(2, 128).T,
         s1.reshape(2, 128).T, b1.reshape(2, 128).T], axis=1,
    ).astype(f)
    fc1wT = (fc1_w.T.reshape(2, 128, K).transpose(1, 0, 2).reshape(128, 2 * K))
    attnc = np.concatenate(
        [np.asarray(inputs["fc2_b"], f)[ks].reshape(128, 1),   # fc2br
         np.full((128, 1), 1.0 / 32.0, f),                     # inv32
         (dy_b[ks, :] / 32.0).astype(f),                       # dybr
         mask,                                                 # mask32
         fc1wT.astype(f)], axis=1,                             # fc1wT
    )
    small4 = np.concatenate(
        [fc2_w[ks, :].T[:, :].astype(f)[:, :128],              # fc2wTr
         np.asarray(inputs["fc1_b"], f).reshape(K, 1)], axis=1,
    )
    small4_pad = np.zeros((128, 129), f)
    small4_pad[:K] = small4
    consts2 = np.concatenate([vec8, attnc, small4_pad], axis=1)  # [128, 435]
    assert consts2.shape == (128, 435), consts2.shape
    consts = {
        "conv1wT": np.ascontiguousarray(conv1_w.T.astype(bf)),      # [C1, C2]
        "consts2": np.ascontiguousarray(consts2),
        "dywsb": dyw_sb,
    }
    return consts


def _make_in_maps(inputs):
    import ml_dtypes

    x = np.ascontiguousarray(
        np.asarray(inputs["x"], np.float32).astype(ml_dtypes.bfloat16)
    )
    consts = _host_prep(inputs)
    in_maps = []
    for core in range(NCORES):
        m = {"x_sh": np.ascontiguousarray(x[core * BL : (core + 1) * BL])}
        m.update(consts)
        in_maps.append(m)
    return in_maps


def kernel(**inputs):
    from concourse.bass_utils import run_bass_kernel_spmd

    if "nc" not in _CACHE:
        _CACHE["nc"] = _build_nc()
    nc = _CACHE["nc"]

    in_maps = _make_in_maps(inputs)

    res = run_bass_kernel_spmd(nc, in_maps, core_ids=list(range(NCORES)))
    globals()["_LAST_RESULTS"] = res
    out = np.concatenate(
        [np.asarray(r["out_sh"]).astype(np.float32) for r in res.results], axis=0
    )
    return out


# revision 23
# speedup vs baseline: 1.0460x; 1.0460x over previous
"""Trainium2 Bass kernel for the dense_cnn problem (1x1 conv + BN + SiLU ->
attention-weighted dynamic 3x3 conv with instance norm), data-parallel over
batch across 8 NeuronCores.

Self-contained: hardcodes all shapes. kernel(**inputs) takes full inputs and
returns the full output.

v7 design (per core, 2 samples, single-pass critical-path minimized):
  The For_i timing loop has an all-engine barrier at its back edge, so each
  iteration is a full drain; the schedule minimizes one pass end-to-end.
  Per-sample pipeline: ph1(A) -> attn(A) -> ph3(A) -> ph5(A) with ALL of
  sample B's pre-work (conv1, attention, weight aggregation) spread through
  ph5(A) at weight-pass granularity (one closure per ~1.6us of PE work,
  each with >=2 passes of cross-engine dependency slack, never in a
  group's last 3 passes) so it hides under ph5(A)'s ~96us of PE time;
  ph5(B) then starts immediately.

  ph1: conv1 matmuls -> ACT Silu into a contiguous stage tile (HW BNStats
       emits exactly one 6-stat group per op, so stats need a flat view) ->
       gpsimd copy into the mu-padded ypad interior.
  Instance norm folded into the dynamic conv (weights scaled by rstd on the
       PSUM->SBUF copy, border ring holds mu, bias correction via tap-sum).
  ph5: 18 accumulating matmuls per flat PSUM tile; drains split between ACT
       (Identity w/ per-partition scale+bias) and DVE (tensor_scalar
       mult+add) so PSUM slots free fast enough for zero PE stalls with 6
       rotating bank-aligned slots; per-group DMA out.
  ACT tables: Exp/Sqrt/Silu preloaded via zero-dep dummy ops placed so the
       1.28us table loads overlap stats/aggr or ph5 slack, not the chain.
  Loop build prefetches the next iteration's inputs at body end (prologue
       DMA covers iteration 1), removing the header DMA wait.
  PSUM budget (8 banks): ph5 cps 6 x [128,512]f32; ph1/attn/ph3 of both
       samples share one sequential 2-slot pool of [128,2,256]f32.
"""

import os

import numpy as np

os.environ.setdefault("BASS_NEVER_TRACE", "1")  # no NTFF hook in this container

EPS = 1e-5
B, C1, C2, K, H, W = 16, 128, 256, 4, 80, 80
NCORES = 8
BL = B // NCORES          # samples per core
RT = 6                    # rows per spatial tile (480 cols <= 512 PSUM bank)
ROWS = [(r, min(RT, H - r)) for r in range(0, H, RT)]   # 13x6 + 1x2
NTILES = len(ROWS)
HP, WP = H + 2, W + 2     # padded 82x82
SGS = [(0, 4), (4, 4), (8, 4), (12, 2)]  # psum accumulation groups over tiles
LDW_OPT = os.environ.get("LDW_OPT", "0") == "1"

_CACHE = {}


def _patch_ldw_opt():
    """Flip walrus's --enable-ldw-opt (redundant load-weight elimination) on
    for our NEFF compiles."""
    import concourse.bass_utils as bu

    if getattr(bu, "_ldw_opt_patched", False):
        return
    orig = bu.run_command

    def patched(argv, **kw):
        if isinstance(argv, list):
            argv = ["--enable-ldw-opt=true" if a == "--enable-ldw-opt=false" else a
                    for a in argv]
        return orig(argv, **kw)

    bu.run_command = patched
    bu._ldw_opt_patched = True


def _build_nc(reps=1, loop_n=None):
    from contextlib import ExitStack

    import concourse.bass as bass
    import concourse.mybir as mybir
    import concourse.tile as tile
    from concourse import bacc

    dt = mybir.dt
    AF = mybir.ActivationFunctionType
    OP = mybir.AluOpType
    f32 = dt.float32
    bf16 = dt.bfloat16

    if LDW_OPT:
        _patch_ldw_opt()

    nc = bacc.Bacc("TRN2", target_bir_lowering=False, debug=False)

    def inp(name, shape, dtype=f32):
        return nc.dram_tensor(name, shape, dtype, kind="ExternalInput").ap()

    x_d = inp("x_sh", [BL, C1, H, W], bf16)
    c1wT_d = inp("conv1wT", [C1, C2], bf16)
    # packed consts, one DMA: cols 0:8 vec8 (bns,bnb,s1,b1 pairs), 8:306 attn
    # consts (fc2br, inv32, dybr, mask32, fc1wT), 306:435 fc2wTr+fc1b (rows 0:4)
    consts_d = inp("consts2", [128, 435])
    # [(k,i_sub), j, tap(9 + tapsum), o]
    dyw_d = inp("dywsb", [128, 8, 10, C2], bf16)
    out_d = nc.dram_tensor("out_sh", [BL, C2, H, W], bf16, kind="ExternalOutput").ap()

    with tile.TileContext(nc) as tc, ExitStack() as ctx:
        pool = lambda name, bufs, **kw: ctx.enter_context(
            tc.tile_pool(name=name, bufs=bufs, **kw)
        )
        const_p = pool("consts", 1)
        ypad_p = pool("ypad", 1)
        aggw_p = pool("aggw", 1)
        stats_p = pool("stats", 1)
        vec_p = pool("vec", 2)
        pers_p = pool("pers", 1)   # per-sample tensors alive until ph5(b)
        bd_p = pool("bd", 1)
        xin_p = pool("xin", 1)
        stage_p = pool("stage", 4)
        ostage_p = pool("ostage", 2)
        # PSUM: 6 + 2 = 8 banks. All of ph1/attn/ph3 for BOTH samples share
        # the 2-slot aps pool (their emission is strictly sequential), so ph5
        # gets 6 rotating slots — drain deadlines stretch to ~2 group-times.
        cps_p = pool("cps", 6, space="PSUM")
        aps_p = pool("aps", 2, space="PSUM")

        # --- const loads; emission order == SP DMA issue order ---
        c1wT = const_p.tile([C1, C2], bf16, tag="c1wT", name="c1wT")
        nc.sync.dma_start(c1wT[:], c1wT_d)
        consts2 = const_p.tile([128, 435], f32, tag="consts2", name="consts2")
        nc.sync.dma_start(consts2[:], consts_d)
        dyw = const_p.tile([128, 8, 10, C2], bf16, tag="dyw", name="dyw")
        for h in range(2):
            nc.sync.dma_start(dyw[:, 4 * h : 4 * h + 4, :, :],
                              dyw_d[:, 4 * h : 4 * h + 4, :, :])

        bns = consts2[:, 0:2]
        bnb = consts2[:, 2:4]
        s1c = consts2[:, 4:6]
        b1c = consts2[:, 6:8]
        fc2br = consts2[:, 8:9]
        inv32 = consts2[:, 9:10]
        dybr = consts2[:, 10 : 10 + C2]
        mask32 = consts2[:, 266:298]
        fc1wT = consts2[:, 298:306]
        fc2wTr = consts2[0:K, 306:434]
        fc1b = consts2[0:K, 434:435]

        ypads = []
        for nm in ("ypadA", "ypadB"):
            yp = ypad_p.tile([128, 2, HP, WP], bf16, tag=nm, name=nm)
            ypads.append(yp)
        epsc = const_p.tile([128, 1], f32, tag="epsc",
                            name="epsc_ldw" if LDW_OPT else "epsc")
        nc.vector.memset(epsc[:], EPS)
        onesT = const_p.tile([128, 128], f32, tag="onesT", name="onesT")
        nc.vector.memset(onesT[:], 1.0)
        ones1 = onesT[0:1, :]
        ones82 = onesT
        dumt = const_p.tile([128, 1], f32, tag="dumt", name="dumt")

        def dummy_act(func):
            """Zero-dependency ACT op to trigger a table load early (overlaps
            the load with other engines' latency instead of stalling a
            consumer)."""
            nc.scalar.activation(dumt[0:1, 0:1], epsc[0:1, 0:1], func)

        def xin_dma(b, xt):
            for h in range(4):
                nc.gpsimd.dma_start(xt[:, 20 * h : 20 * h + 20, :],
                                    x_d[b, :, 20 * h : 20 * h + 20, :])

        def conv_tiles(b, xt, stats, mv, tiles, aggr_after=False):
            """conv1 matmul -> silu into a contiguous stage tile (bn_stats
            needs a flat view; HW emits exactly one 6-stat group per op) ->
            gpsimd copy into the ypad interior."""
            yp = ypads[b]
            for c, t in tiles:
                r0, rn = ROWS[t]
                ps = aps_p.tile([128, 2, 256], f32, tag="aps", name="cps1")
                psf = ps.rearrange("p a b -> p (a b)")
                nc.tensor.matmul(
                    psf[:, : rn * W], c1wT[:, 128 * c : 128 * (c + 1)],
                    xt[:, r0 : r0 + rn, :], start=True, stop=True,
                )
                st = stage_p.tile([128, RT, W], bf16, tag="stage", name="stage")
                nc.scalar.activation(
                    st[:, :rn, :], psf[:, : rn * W],
                    AF.Silu, bias=bnb[:, c : c + 1], scale=bns[:, c : c + 1],
                )
                nc.vector.bn_stats(
                    stats[c][:, t, :],
                    st[:, :rn, :].rearrange("p a b -> p (a b)"),
                )
                nc.gpsimd.tensor_copy(
                    yp[:, c, r0 + 1 : r0 + rn + 1, 1 : W + 1], st[:, :rn, :]
                )
            if aggr_after:
                for c in range(2):
                    nc.vector.bn_aggr(mv[:, c, :], stats[c][:])

        def borders(b, mv):
            """Fill ypad border ring with mu_i so padding is zero in
            normalized space."""
            yp = ypads[b]
            for c in range(2):
                mu = mv[:, c, 0:1]
                nc.vector.tensor_scalar_mul(yp[:, c, 0, :], ones82[:, 0:WP], mu)
                nc.vector.tensor_scalar_mul(yp[:, c, HP - 1, :], ones82[:, 0:WP], mu)
                nc.vector.tensor_scalar_mul(yp[:, c, 1 : H + 1, 0], ones82[:, 0:H], mu)
                nc.vector.tensor_scalar_mul(yp[:, c, 1 : H + 1, WP - 1], ones82[:, 0:H], mu)

        def attn_fc1(b, mv, pers):
            """fc1 matmul + relu. Exp table must already be loaded/loading."""
            aT = aps_p.tile([128, 2, 256], f32, tag="aps", name="a_ps")
            a_ps = aT[0:K, 0, 0:1]
            for c in range(2):
                nc.tensor.matmul(
                    a_ps, fc1wT[:, K * c : K * (c + 1)], mv[:, c, 0:1],
                    start=(c == 0), stop=(c == 1),
                )
            a_sb = vec_p.tile([128, 1], f32, tag="a_sb", name="a_sb")
            nc.scalar.activation(a_sb[0:K, 0:1], a_ps, AF.Relu, bias=fc1b[:])
            pers["a_sb"] = a_sb

        def attn_fc2(b, pers):
            zT = aps_p.tile([128, 2, 256], f32, tag="aps", name="z_ps")
            z_ps = zT[:, 0, 0:1]
            nc.tensor.matmul(z_ps, fc2wTr[:], pers["a_sb"][0:K, 0:1],
                             start=True, stop=True)
            e_bc = pers["e_bc"]
            nc.scalar.activation(e_bc[:], z_ps, AF.Exp, bias=fc2br[:])
            nc.vector.tensor_scalar_mul(pers["bd"][:], mask32[:], e_bc[:, 0:1])

        def attn_S(b, pers):
            sT = aps_p.tile([128, 2, 256], f32, tag="aps", name="S_ps")
            S_ps = sT[0:1, 0, 0:1]
            nc.tensor.matmul(S_ps, inv32[:, 0:1], pers["e_bc"][:, 0:1],
                             start=True, stop=True)
            rS = vec_p.tile([128, 1], f32, tag="rS", name="rS")
            nc.vector.reciprocal(rS[0:1, 0:1], S_ps)
            pers["rS"] = rS

        def attn_rb(b, pers):
            rT = aps_p.tile([128, 2, 256], f32, tag="aps", name="rb_ps")
            rb_ps = rT[:, 0, 0:1]
            nc.tensor.matmul(rb_ps, ones1[:], pers["rS"][0:1, 0:1],
                             start=True, stop=True)
            s1S = pers["s1S"]
            for oc in range(2):
                nc.vector.tensor_scalar_mul(s1S[:, oc : oc + 1], rb_ps,
                                            s1c[:, oc : oc + 1])

        def attn_aggb(b, pers):
            gT = aps_p.tile([128, 2, 256], f32, tag="aps", name="aggb_ps")
            aggb_ps = gT[:, 0:2, 0:1]
            for oc in range(2):
                nc.tensor.matmul(
                    aggb_ps[:, oc, :], dybr[:, 128 * oc : 128 * (oc + 1)],
                    pers["e_bc"][:, 0:1], start=True, stop=True,
                )
            aggb_sb = vec_p.tile([128, 2], f32, tag="aggb_sb", name="aggb_sb")
            nc.vector.tensor_copy(aggb_sb[:], aggb_ps[:, :, 0])
            pers["aggb_sb"] = aggb_sb
            dummy_act(AF.Sqrt)

        def attn_rstd(b, mv, pers):
            """Sqrt table must be loaded/loading (dummy in attn_aggb)."""
            std = vec_p.tile([128, 2], f32, tag="std", name="std")
            nc.scalar.activation(std[:], mv[:, :, 1], AF.Sqrt, bias=epsc[:])
            rstd = pers["rstd"]
            nc.vector.reciprocal(rstd[:], std[:])
            nc.vector.tensor_copy(pers["mean_bf"][:], mv[:, :, 0])

        def attention(b, mv, pers):
            attn_fc1(b, mv, pers)
            attn_fc2(b, pers)
            attn_S(b, pers)
            attn_rb(b, pers)
            attn_aggb(b, pers)
            attn_rstd(b, mv, pers)

        def ph3_pair(b, pers, c, t0):
            """One tap-pair of aggW = blockdiag(e) @ dyw, scaled by rstd on
            the PSUM->SBUF copy."""
            aggW, bd, rstd = pers["aggW"], pers["bd"], pers["rstd"]
            agps = aps_p.tile([128, 2, C2], f32, tag="aps", name="agps")
            for jj in range(4):
                nc.tensor.matmul(
                    agps[32 * jj : 32 * jj + 32, :, :],
                    bd[:],
                    dyw[:, 4 * c + jj, t0 : t0 + 2, :],
                    start=True, stop=True,
                    tile_position=(0, 32 * jj),
                )
            nc.vector.tensor_scalar_mul(
                aggW[:, c, t0 : t0 + 2, :], agps[:],
                rstd[:, c : c + 1],
            )

        def ph3(b, pers, cs=(0, 1)):
            for c in cs:
                for t0 in range(0, 10, 2):
                    ph3_pair(b, pers, c, t0)

        def ph3_bias(b, pers):
            """B_o = sum_i mu_i * (rstd-scaled tap-sum), then
            cbias = (aggb - B) * s1S + b1."""
            aggW, mean_bf = pers["aggW"], pers["mean_bf"]
            bT = aps_p.tile([128, 2, 256], f32, tag="aps", name="B_ps")
            B_ps = bT[:, 0:2, 0:1]
            for oc in range(2):
                for ci in range(2):
                    nc.tensor.matmul(
                        B_ps[:, oc, :],
                        aggW[:, ci, 9, 128 * oc : 128 * (oc + 1)],
                        mean_bf[:, ci : ci + 1],
                        start=(ci == 0), stop=(ci == 1),
                    )
            B_sb = vec_p.tile([128, 2], f32, tag="B_sb", name="B_sb")
            nc.vector.tensor_copy(B_sb[:], B_ps[:, :, 0])
            t4 = vec_p.tile([128, 2], f32, tag="t4", name="t4")
            nc.vector.tensor_sub(t4[:], pers["aggb_sb"][:], B_sb[:])
            cbias = pers["cbias"]
            for oc in range(2):
                nc.vector.scalar_tensor_tensor(
                    cbias[:, oc : oc + 1], t4[:, oc : oc + 1],
                    pers["s1S"][:, oc : oc + 1], b1c[:, oc : oc + 1],
                    op0=OP.mult, op1=OP.add,
                )

        def ph5(b, pers, fill_map=None):
            """dynamic 3x3 conv; drains split ACT/DVE; per-group DMA out.
            fill_map maps a global weight-pass index (0..143: 18 passes per
            group, 8 groups oc-major) to closures emitted after that pass —
            used to spread the other sample's pre-work through this phase
            with enough dependency slack that the PE never stalls."""
            aggW, s1S, cbias = pers["aggW"], pers["s1S"], pers["cbias"]
            fill_map = fill_map or {}
            pi = 0
            for oc in range(2):
                stage = ostage_p.tile([128, H, W], bf16, tag="ostage", name="ostage")
                for g0, gn in SGS:
                    pss = [cps_p.tile([128, 512], f32, tag="cps", name="cps")
                           for _ in range(gn)]
                    first, last = (0, 0, 0), (1, 2, 2)
                    for c in range(2):
                        for ty in range(3):
                            for tx in range(3):
                                lhsT = aggW[:, c, 3 * ty + tx,
                                            128 * oc : 128 * (oc + 1)]
                                st = (c, ty, tx) == first
                                sp = (c, ty, tx) == last
                                for ti in range(gn):
                                    r0, rn = ROWS[g0 + ti]
                                    rhs = ypads[b][:, c, r0 + ty : r0 + ty + rn,
                                                   tx : tx + W]
                                    nc.tensor.matmul(
                                        pss[ti][:, : rn * W], lhsT, rhs,
                                        start=st, stop=sp,
                                    )
                                for f in fill_map.get(pi, ()):
                                    f()
                                pi += 1
                    for ti in range(gn):
                        r0, rn = ROWS[g0 + ti]
                        if ti % 2 == 0:
                            nc.scalar.activation(
                                stage[:, r0 : r0 + rn, :], pss[ti][:, : rn * W],
                                AF.Identity,
                                bias=cbias[:, oc : oc + 1],
                                scale=s1S[:, oc : oc + 1],
                            )
                        else:
                            nc.vector.tensor_scalar(
                                stage[:, r0 : r0 + rn, :], pss[ti][:, : rn * W],
                                s1S[:, oc : oc + 1], cbias[:, oc : oc + 1],
                                op0=OP.mult, op1=OP.add,
                            )
                    gr0 = ROWS[g0][0]
                    grn = sum(ROWS[g0 + i][1] for i in range(gn))
                    nc.gpsimd.dma_start(
                        out_d[b, 128 * oc : 128 * (oc + 1), gr0 : gr0 + grn, :],
                        stage[:, gr0 : gr0 + grn, :],
                    )

        def make_pers(b):
            sfx = "AB"[b]
            pers = {
                "aggW": aggw_p.tile([128, 2, 10, C2], bf16, tag=f"aggW{sfx}",
                                    name=f"aggW{sfx}"),
                "rstd": pers_p.tile([128, 2], f32, tag=f"rstd{sfx}", name=f"rstd{sfx}"),
                "mean_bf": pers_p.tile([128, 2], bf16, tag=f"meanbf{sfx}",
                                       name=f"meanbf{sfx}"),
                "e_bc": pers_p.tile([128, 1], f32, tag=f"ebc{sfx}", name=f"ebc{sfx}"),
                "bd": bd_p.tile([128, 32], bf16, tag=f"bd{sfx}", name=f"bd{sfx}"),
                "s1S": pers_p.tile([128, 2], f32, tag=f"s1S{sfx}", name=f"s1S{sfx}"),
                "cbias": pers_p.tile([128, 2], f32, tag=f"cbias{sfx}",
                                     name=f"cbias{sfx}"),
            }
            return pers

        # --- schedule: one barrier-bounded pass over 2 samples ---
        def body(prefetch=False, xts=None):
            for _rep in range(reps):
                statsA = [stats_p.tile([128, NTILES, 6], f32, tag=f"statsA{c}",
                                       name=f"statsA{c}") for c in range(2)]
                statsB = [stats_p.tile([128, NTILES, 6], f32, tag=f"statsB{c}",
                                       name=f"statsB{c}") for c in range(2)]
                mvA = vec_p.tile([128, 2, 2], f32, tag="mvA", name="mvA")
                mvB = vec_p.tile([128, 2, 2], f32, tag="mvB", name="mvB")
                persA = make_pers(0)
                persB = make_pers(1)
                if xts is None:
                    xtA = xin_p.tile([C1, H, W], bf16, tag="xtA", name="xtA")
                    xtB = xin_p.tile([C1, H, W], bf16, tag="xtB", name="xtB")
                else:
                    xtA, xtB = xts
                if not prefetch:
                    xin_dma(0, xtA)
                    xin_dma(1, xtB)

                # ph1(A): full conv1+silu+stats, then aggr
                tilesA = [(c, t) for c in range(2) for t in range(NTILES)]
                conv_tiles(0, xtA, statsA, mvA, tilesA, aggr_after=True)
                dummy_act(AF.Exp)    # load exp table during stats/aggr window
                attention(0, mvA, persA)
                borders(0, mvA)
                ph3(0, persA)
                ph3_bias(0, persA)
                dummy_act(AF.Silu)   # load silu table back for sample B

                # sample B pre-work spread through ph5(A) at weight-pass
                # granularity (one pass ~= 0.8us of PE work); each closure's
                # cross-engine deps resolve >=2 passes earlier, so the PE
                # stream never blocks on ACT/DVE.
                tB = [(c, t) for c in range(2) for t in range(NTILES)]
                fill = {}

                def at(p, f):
                    fill.setdefault(p, []).append(f)

                def fB(i):
                    def f():
                        conv_tiles(1, xtB, statsB, mvB, [tB[i]],
                                   aggr_after=(i == 27))
                        if i == 27:
                            dummy_act(AF.Exp)
                    return f

                # Keep closures out of each group's last 3 passes: a closure's
                # ACT/DVE ops emitted at a group tail land in the engine
                # streams ahead of that group's PSUM drains and delay them,
                # stalling the next group's slot reuse.
                slots = [p for p in range(144) if p % 18 < 15 and p % 2 == 1]
                for i in range(28):
                    at(slots[i], fB(i))                      # within oc0 groups
                at(slots[29], lambda: attn_fc1(1, mvB, persB))
                at(slots[31], lambda: attn_fc2(1, persB))
                at(slots[33], lambda: attn_S(1, persB))
                at(slots[35], lambda: attn_rb(1, persB))
                at(slots[37], lambda: attn_aggb(1, persB))
                at(slots[39], lambda: (attn_rstd(1, mvB, persB), borders(1, mvB)))
                for i in range(5):
                    at(slots[41 + i],
                       (lambda t0: lambda: ph3_pair(1, persB, 0, t0))(2 * i))
                for i in range(5):
                    at(slots[46 + i],
                       (lambda t0: lambda: ph3_pair(1, persB, 1, t0))(2 * i))

                def f_bias():
                    ph3_bias(1, persB)
                    dummy_act(AF.Silu)

                at(slots[52], f_bias)

                ph5(0, persA, fill_map=fill)
                ph5(1, persB)
                if prefetch:
                    # next iteration's inputs land during ph5; the input data
                    # is identical every iteration, so this is exact
                    xin_dma(0, xtA)
                    xin_dma(1, xtB)

        if loop_n is None:
            body()
        else:
            # prologue: first iteration's inputs + silu table; the body reads
            # and end-of-body-rewrites these same tiles each iteration
            pxtA = xin_p.tile([C1, H, W], bf16, tag="xtA", name="xtA")
            pxtB = xin_p.tile([C1, H, W], bf16, tag="xtB", name="xtB")
            xin_dma(0, pxtA)
            xin_dma(1, pxtB)
            dummy_act(AF.Silu)
            with tc.For_i(0, loop_n, 1):
                body(prefetch=True, xts=(pxtA, pxtB))

    if not nc.is_finalized():
        nc.finalize()
    return nc


def _host_prep(inputs):
    f = np.float32
    conv1_w = np.asarray(inputs["conv1_w"], f)
    bns = (np.asarray(inputs["bn_g"], f) / np.sqrt(np.asarray(inputs["bn_v"], f) + EPS))
    bnb = np.asarray(inputs["bn_b"], f) - np.asarray(inputs["bn_m"], f) * bns
    s1 = (np.asarray(inputs["bn1_g"], f) / np.sqrt(np.asarray(inputs["bn1_v"], f) + EPS))
    b1 = np.asarray(inputs["bn1_b"], f) - np.asarray(inputs["bn1_m"], f) * s1
    fc1_w = np.asarray(inputs["fc1_w"], f)
    fc2_w = np.asarray(inputs["fc2_w"], f)
    dy_w = np.asarray(inputs["dy_w"], f)
    dy_b = np.asarray(inputs["dy_b"], f)

    import ml_dtypes

    bf = ml_dtypes.bfloat16
    t = dy_w.transpose(0, 2, 3, 4, 1)              # [k, i, ty, tx, o]
    t = t.reshape(K, 8, 32, 3, 3, C2)              # [k, j, i_sub, ty, tx, o]
    t9 = t.transpose(0, 2, 1, 3, 4, 5).reshape(128, 8, 9, C2)
    dyw10 = np.concatenate([t9, t9.sum(axis=2, keepdims=True)], axis=2)
    dyw_sb = np.ascontiguousarray(dyw10.astype(bf))
    mask = np.zeros((128, 32), f)
    mask[np.arange(128), np.arange(128) % 32] = 1.0
    ks = np.arange(128) // 32
    vec8 = np.concatenate(
        [bns.reshape(2, 128).T, bnb.reshape(2, 128).T,
         s1.reshape(2, 128).T, b1.reshape(2, 128).T], axis=1,
    ).astype(f)
    fc1wT = (fc1_w.T.reshape(2, 128, K).transpose(1, 0, 2).reshape(128, 2 * K))
    attnc = np.concatenate(
        [np.asarray(inputs["fc2_b"], f)[ks].reshape(128, 1),   # fc2br
         np.full((128, 1), 1.0 / 32.0, f),                     # inv32
         (dy_b[ks, :] / 32.0).astype(f),                       # dybr
         mask,                                                 # mask32
         fc1wT.astype(f)], axis=1,                             # fc1wT
    )
    small4 = np.concatenate(
        [fc2_w[ks, :].T[:, :].astype(f)[:, :128],              # fc2wTr
         np.asarray(inputs["fc1_b"], f).reshape(K, 1)], axis=1,
    )
    small4_pad = np.zeros((128, 129), f)
    small4_pad[:K] = small4
    consts2 = np.concatenate([vec8, attnc, small4_pad], axis=1)  # [128, 435]
    assert consts2.shape == (128, 435), consts2.shape
    consts = {
        "conv1wT": np.ascontiguousarray(conv1_w.T.astype(bf)),      # [C1, C2]
        "consts2": np.ascontiguousarray(consts2),
        "dywsb": dyw_sb,
    }
    return consts


def _make_in_maps(inputs):
    import ml_dtypes

    x = np.ascontiguousarray(
        np.asarray(inputs["x"], np.float32).astype(ml_dtypes.bfloat16)
    )
    consts = _host_prep(inputs)
    in_maps = []
    for core in range(NCORES):
        m = {"x_sh": np.ascontiguousarray(x[core * BL : (core + 1) * BL])}
        m.update(consts)
        in_maps.append(m)
    return in_maps


def kernel(**inputs):
    from concourse.bass_utils import run_bass_kernel_spmd

    if "nc" not in _CACHE:
        _CACHE["nc"] = _build_nc()
    nc = _CACHE["nc"]

    in_maps = _make_in_maps(inputs)

    res = run_bass_kernel_spmd(nc, in_maps, core_ids=list(range(NCORES)))
    globals()["_LAST_RESULTS"] = res
    out = np.concatenate(
        [np.asarray(r["out_sh"]).astype(np.float32) for r in res.results], axis=0
    )
    return out
